# revision 1
# baseline (speedup 1.0000x reference)
"""Trainium2 Bass kernel for nn_Encoder_61830349193463 (retrieval_knn).

Strategy (data-parallel over src rows, 8 NeuronCores):
  - Each core gets a 2048-row shard of src; anchors + weights replicated.
  - kNN distances via PE matmul. Ranking needs ~fp32 precision (5th/6th
    neighbour gaps go down to 8e-5), but fp32 matmul is 4x slow on PE, so
    the dot products use a 3-term split-bf16 decomposition:
        x = h + l (bf16 hi/lo);  s.a ~= sh.ah + sh.al + sl.ah
    accumulated in fp32 PSUM (error ~5e-5, matches fp64 top-5 selection).
  - sim = dot - 0.5*||a||^2 (row-constant ||s||^2 dropped; ranking-equivalent).
    The ||a||^2 term is applied on the Vector engine while draining PSUM.
  - top-8 per row via DVE max8/max_index over m-quarters + small merge.
  - top-5 anchor rows gathered with indirect DMA (fp32 exact), mean on DVE.
  - Dense chain (linear_dim, fusion, BN1, MLP, BN2, decoder BN3+tanh) runs
    feature-major ([feature, n] layout) so BatchNorm scale/bias fuse into
    single ScalarEngine activation passes; batch stats are summed locally
    and AllReduced across the 8 cores (3 tiny collectives).
  - Final [512, 2048] -> [2048, 512] transpose on PE.
"""

import numpy as np

import concourse.bacc as bacc
import concourse.bass as bass
import concourse.mybir as mybir
import concourse.tile as tile
from concourse.bass import IndirectOffsetOnAxis
from concourse.bass_utils import run_bass_kernel_spmd
from concourse.masks import make_identity
import ml_dtypes

F32 = mybir.dt.float32
BF16 = mybir.dt.bfloat16
U32 = mybir.dt.uint32
AF = mybir.ActivationFunctionType
OP = mybir.AluOpType
P = 128

# problem sizes (hardcoded per contract)
N_FULL, M, D, F = 16384, 8192, 512, 2048
N_CORES = 8
K = 5
EPS = 1e-5


def build_kernel(ns=N_FULL // N_CORES, m=M, d=D, f=F, n_cores=N_CORES,
                 mc_free=512, q_div=4):
    """Build the SPMD Bass module. ns/m/d/f sizes are per-core."""
    DC = d // P          # contraction chunks of the d dim (4)
    FC = f // P          # chunks of the hidden dim (16)
    T = ns // P          # n-tiles per core (16)
    nbf = min(mc_free, ns)
    NB = ns // nbf       # n blocks of 512 for phase-B matmuls (4)
    MQ = m // q_div      # m-quarter size (2048)
    QC = MQ // mc_free   # 512-chunks per quarter (4)
    NTOT = float(ns * n_cores)

    nc = bacc.Bacc("TRN2", target_bir_lowering=False, debug=False,
                   num_devices=n_cores)

    def param(name, shape, dt=F32):
        return nc.declare_dram_parameter(name, list(shape), dt, isOutput=False)

    srcT_h = param("srcT_h", [d, ns], BF16)
    srcT_l = param("srcT_l", [d, ns], BF16)
    anchT_h = param("anchT_h", [d, m], BF16)
    anchT_l = param("anchT_l", [d, m], BF16)
    anchor = param("anchor", [m, d], F32)          # natural, for the gather
    am2b = param("am2b", [P, m], F32)              # 0.5*||a||^2 bcast to 128 rows
    wdim = param("wdim", [d, d], BF16)             # pre-scaled by 1/K
    wfus = param("wfus", [2 * d, d], BF16)
    we1 = param("we1", [d, f], BF16)
    we2 = param("we2", [f, d], BF16)
    wd = param("wd", [d, d], BF16)
    bdim = param("bdim", [P, DC])
    bfus = param("bfus", [P, DC])
    be1 = param("be1", [P, FC])
    be2 = param("be2", [P, DC])
    bd = param("bd", [P, DC])
    g1 = param("g1", [P, DC]); bt1 = param("bt1", [P, DC])
    g2 = param("g2", [P, DC]); bt2 = param("bt2", [P, DC])
    gd = param("gd", [P, DC]); btd = param("btd", [P, DC])
    out = nc.declare_dram_parameter("out", [ns, d], F32, isOutput=True)

    # internal DRAM for the three BN-stat AllReduces
    cc_in = [nc.dram_tensor(f"cc{i}_in", [P, 2 * DC], F32) for i in range(3)]
    cc_space = "Shared" if n_cores > 4 else "Local"
    cc_out = [nc.dram_tensor(f"cc{i}_out", [P, 2 * DC], F32,
                             addr_space=cc_space) for i in range(3)]
    groups = [list(range(n_cores))]

    with tile.TileContext(nc) as tc:
        with (
            tc.tile_pool(name="persist", bufs=1) as pp,
            tc.tile_pool(name="wpool", bufs=1) as wp,
        ):
            ident = pp.tile([P, P], F32, name="ident")
            make_identity(nc, ident[:])

            # ---- resident source splits ----
            sTh = []
            sTl = []
            for c in range(DC):
                th = pp.tile([P, ns], BF16, tag=f"sTh{c}", name=f"sTh{c}")
                tl = pp.tile([P, ns], BF16, tag=f"sTl{c}", name=f"sTl{c}")
                nc.sync.dma_start(out=th[:], in_=srcT_h[c * P:(c + 1) * P, :])
                nc.sync.dma_start(out=tl[:], in_=srcT_l[c * P:(c + 1) * P, :])
                sTh.append(th)
                sTl.append(tl)

            # neighbour-mean output, feature-major bf16
            neighT = [pp.tile([P, ns], BF16, tag=f"nT{c}", name=f"nT{c}") for c in range(DC)]

            # per-tile top-8 candidates from each quarter (values + indices)
            vcand = [pp.tile([P, 8 * q_div], F32, tag=f"vc{t}", name=f"vc{t}") for t in range(T)]
            icand = [pp.tile([P, 8 * q_div], F32, tag=f"ic{t}", name=f"ic{t}") for t in range(T)]

            # ================= PHASE A: kNN =================
            with (
                tc.tile_pool(name="aq", bufs=2) as aq_pool,
                tc.tile_pool(name="am2q", bufs=2) as am2_pool,
                tc.tile_pool(name="simq", bufs=2) as sim_pool,
                tc.tile_pool(name="dps", bufs=4, space="PSUM") as dps,
                tc.tile_pool(name="tops", bufs=4) as tops,
            ):
                for q in range(q_div):
                    aqh = [aq_pool.tile([P, MQ], BF16, tag=f"aqh{c}", name=f"aqh{c}")
                           for c in range(DC)]
                    aql = [aq_pool.tile([P, MQ], BF16, tag=f"aql{c}", name=f"aql{c}")
                           for c in range(DC)]
                    for c in range(DC):
                        nc.sync.dma_start(
                            out=aqh[c][:],
                            in_=anchT_h[c * P:(c + 1) * P, q * MQ:(q + 1) * MQ])
                        nc.sync.dma_start(
                            out=aql[c][:],
                            in_=anchT_l[c * P:(c + 1) * P, q * MQ:(q + 1) * MQ])
                    am2q = am2_pool.tile([P, MQ], F32, tag="am2q", name="am2q")
                    nc.sync.dma_start(out=am2q[:],
                                      in_=am2b[:, q * MQ:(q + 1) * MQ])

                    for t in range(T):
                        simq = sim_pool.tile([P, MQ], F32, tag="simq", name="simq")
                        for mc in range(QC):
                            ps = dps.tile([P, mc_free], F32, name="dps")
                            n_sl = slice(t * P, (t + 1) * P)
                            m_sl = slice(mc * mc_free, (mc + 1) * mc_free)
                            for c in range(DC):
                                nc.tensor.matmul(ps[:], sTh[c][:, n_sl],
                                                 aqh[c][:, m_sl],
                                                 start=(c == 0), stop=False)
                            for c in range(DC):
                                nc.tensor.matmul(ps[:], sTh[c][:, n_sl],
                                                 aql[c][:, m_sl],
                                                 start=False, stop=False)
                            for c in range(DC):
                                nc.tensor.matmul(ps[:], sTl[c][:, n_sl],
                                                 aqh[c][:, m_sl],
                                                 start=False, stop=(c == DC - 1))
                            # sim = dot - 0.5*||a||^2, drained psum->sbuf
                            nc.vector.scalar_tensor_tensor(
                                out=simq[:, m_sl], in0=ps[:], scalar=1.0,
                                in1=am2q[:, m_sl], op0=OP.mult, op1=OP.subtract)
                        v8 = tops.tile([P, 8], F32, tag="v8", name="v8")
                        nc.vector.max(out=v8[:], in_=simq[:])
                        i8 = tops.tile([P, 8], U32, tag="i8", name="i8")
                        nc.vector.max_index(out=i8[:], in_max=v8[:],
                                            in_values=simq[:])
                        nc.vector.tensor_copy(vcand[t][:, q * 8:(q + 1) * 8],
                                              v8[:])
                        # local -> global m index, carried in fp32 (exact)
                        nc.vector.tensor_scalar(
                            out=icand[t][:, q * 8:(q + 1) * 8], in0=i8[:],
                            scalar1=float(q * MQ), scalar2=None, op0=OP.add)

            # ---- merge quarters, gather top-5 anchors, mean, transpose ----
            with (
                tc.tile_pool(name="mrg", bufs=4) as mrg,
                tc.tile_pool(name="gat", bufs=2) as gat,
                tc.tile_pool(name="tps", bufs=2, space="PSUM") as tpsp,
            ):
                NQ = 8 * q_div
                for t in range(T):
                    g8 = mrg.tile([P, 8], F32, tag="g8", name="g8")
                    nc.vector.max(out=g8[:], in_=vcand[t][:])
                    # match each of the global top-8 values back to its index
                    eqm = mrg.tile([P, 8 * NQ], F32, tag="eqm", name="eqm")
                    nc.vector.tensor_tensor(
                        out=eqm[:].rearrange("p (a b) -> p a b", a=8),
                        in0=vcand[t][:].rearrange("p (a q) -> p a q", a=1).to_broadcast(
                            [P, 8, NQ]),
                        in1=g8[:].rearrange("p (a o) -> p a o", o=1).to_broadcast(
                            [P, 8, NQ]),
                        op=OP.is_equal)
                    prod = mrg.tile([P, 8 * NQ], F32, tag="prod", name="prod")
                    nc.vector.tensor_tensor(
                        out=prod[:].rearrange("p (a b) -> p a b", a=8),
                        in0=eqm[:].rearrange("p (a b) -> p a b", a=8),
                        in1=icand[t][:].rearrange("p (a q) -> p a q", a=1).to_broadcast(
                            [P, 8, NQ]),
                        op=OP.mult)
                    idx8f = mrg.tile([P, 8], F32, tag="idx8f", name="idx8f")
                    nc.vector.tensor_reduce(
                        out=idx8f[:],
                        in_=prod[:].rearrange("p (a b) -> p a b", a=8),
                        axis=mybir.AxisListType.X, op=OP.add)
                    idx8 = mrg.tile([P, 8], U32, tag="idx8", name="idx8")
                    nc.vector.tensor_copy(idx8[:], idx8f[:])

                    G = gat.tile([P, K * d], F32, tag="G", name="G")
                    for k in range(K):
                        nc.gpsimd.indirect_dma_start(
                            out=G[:, k * d:(k + 1) * d], out_offset=None,
                            in_=anchor[:],
                            in_offset=IndirectOffsetOnAxis(
                                ap=idx8[:, k:k + 1], axis=0))
                    meanv = gat.tile([P, d], F32, tag="meanv", name="meanv")
                    nc.vector.tensor_reduce(
                        out=meanv[:],
                        in_=G[:].rearrange("p (k e) -> p e k", k=K),
                        axis=mybir.AxisListType.X, op=OP.add)
                    tps = tpsp.tile([P, d], F32, name="tps")
                    for j in range(DC):
                        nc.tensor.transpose(
                            out=tps[:, j * P:(j + 1) * P],
                            in_=meanv[:, j * P:(j + 1) * P], identity=ident[:])
                    for j in range(DC):
                        nc.scalar.copy(neighT[j][:, t * P:(t + 1) * P],
                                       tps[:, j * P:(j + 1) * P])

            # ================= PHASE B: dense chain =================
            def load_w(t_dram, rows, cols, tag):
                tiles = []
                for c in range(rows // P):
                    w = wp.tile([P, cols], BF16, tag=f"{tag}{c}", name=f"{tag}{c}")
                    nc.sync.dma_start(out=w[:], in_=t_dram[c * P:(c + 1) * P, :])
                    tiles.append(w)
                return tiles

            wdim_t = load_w(wdim, d, d, "wdim")
            wfus_t = load_w(wfus, 2 * d, d, "wfus")
            we1_t = load_w(we1, d, f, "we1")
            we2_t = load_w(we2, f, d, "we2")
            wd_t = load_w(wd, d, d, "wd")

            bias_t = {}
            for name, t_dram, cols in [
                    ("bdim", bdim, DC), ("bfus", bfus, DC), ("be1", be1, FC),
                    ("be2", be2, DC), ("bd", bd, DC), ("g1", g1, DC),
                    ("bt1", bt1, DC), ("g2", g2, DC), ("bt2", bt2, DC),
                    ("gd", gd, DC), ("btd", btd, DC)]:
                bt_ = wp.tile([P, cols], F32, tag=name, name=name)
                nc.sync.dma_start(out=bt_[:], in_=t_dram[:, :])
                bias_t[name] = bt_

            with (
                tc.tile_pool(name="act", bufs=1) as ap_,
                tc.tile_pool(name="mlp", bufs=1) as mp_,
                tc.tile_pool(name="bps", bufs=4, space="PSUM") as bps,
                tc.tile_pool(name="stat", bufs=1) as stp,
                tc.tile_pool(name="dram", bufs=1, space="DRAM") as _dp,
            ):
                amp_ctx = tc.tile_pool(name="amap", bufs=1)
                amp = amp_ctx.__enter__()
                amapT = [amp.tile([P, ns], BF16, tag=f"amap{c}", name=f"amap{c}")
                         for c in range(DC)]
                for nb in range(NB):
                    n_sl = slice(nb * nbf, (nb + 1) * nbf)
                    for fc in range(DC):
                        ps = bps.tile([P, nbf], F32, tag="psB", name="psB")
                        for c in range(DC):
                            nc.tensor.matmul(
                                ps[:], wdim_t[c][:, fc * P:(fc + 1) * P],
                                neighT[c][:, n_sl],
                                start=(c == 0), stop=(c == DC - 1))
                        nc.scalar.activation(amapT[fc][:, n_sl], ps[:],
                                             AF.Identity,
                                             bias=bias_t["bdim"][:, fc:fc + 1])

                combraw = [ap_.tile([P, ns], BF16, tag=f"craw{c}", name=f"craw{c}")
                           for c in range(DC)]
                for nb in range(NB):
                    n_sl = slice(nb * nbf, (nb + 1) * nbf)
                    for fc in range(DC):
                        ps = bps.tile([P, nbf], F32, tag="psB", name="psB")
                        for c in range(2 * DC):
                            rhs = sTh[c][:, n_sl] if c < DC else \
                                amapT[c - DC][:, n_sl]
                            nc.tensor.matmul(
                                ps[:], wfus_t[c][:, fc * P:(fc + 1) * P], rhs,
                                start=(c == 0), stop=(c == 2 * DC - 1))
                        nc.scalar.activation(combraw[fc][:, n_sl], ps[:],
                                             AF.Identity,
                                             bias=bias_t["bfus"][:, fc:fc + 1])

                amp_ctx.__exit__(None, None, None)

                def bn_stats(tiles, idx):
                    st = stp.tile([P, 2 * DC], F32, tag=f"st{idx}", name=f"st{idx}")
                    scr = stp.tile([P, ns], BF16, tag="sq_scratch", name="sq_scratch")
                    for c in range(DC):
                        nc.vector.tensor_reduce(out=st[:, c:c + 1],
                                                in_=tiles[c][:],
                                                axis=mybir.AxisListType.X,
                                                op=OP.add)
                        nc.scalar.activation(scr[:], tiles[c][:], AF.Square,
                                             accum_out=st[:, DC + c:DC + c + 1])
                    nc.sync.dma_start(out=cc_in[idx][:], in_=st[:])
                    nc.gpsimd.collective_compute(
                        "AllReduce", OP.add, replica_groups=groups,
                        ins=[cc_in[idx].ap()], outs=[cc_out[idx].ap()])
                    gst = stp.tile([P, 2 * DC], F32, tag=f"gst{idx}", name=f"gst{idx}")
                    nc.sync.dma_start(out=gst[:], in_=cc_out[idx][:])
                    # mu, var=E[x^2]-mu^2, s=g/sqrt(var+eps), t=beta-mu*s
                    mu = stp.tile([P, DC], F32, tag=f"mu{idx}", name=f"mu{idx}")
                    nc.vector.tensor_scalar(out=mu[:], in0=gst[:, :DC],
                                            scalar1=1.0 / NTOT, scalar2=None,
                                            op0=OP.mult)
                    musq = stp.tile([P, DC], F32, tag=f"musq{idx}", name=f"musq{idx}")
                    nc.vector.tensor_tensor(out=musq[:], in0=mu[:], in1=mu[:],
                                            op=OP.mult)
                    var = stp.tile([P, DC], F32, tag=f"var{idx}", name=f"var{idx}")
                    nc.vector.scalar_tensor_tensor(
                        out=var[:], in0=gst[:, DC:], scalar=1.0 / NTOT,
                        in1=musq[:], op0=OP.mult, op1=OP.subtract)
                    sd = stp.tile([P, DC], F32, tag=f"sd{idx}", name=f"sd{idx}")
                    nc.vector.tensor_scalar(out=sd[:], in0=var[:], scalar1=EPS,
                                            scalar2=None, op0=OP.add)
                    nc.scalar.sqrt(sd[:], sd[:])
                    rs = stp.tile([P, DC], F32, tag=f"rs{idx}", name=f"rs{idx}")
                    nc.vector.reciprocal(rs[:], sd[:])
                    return mu, rs

                def bn_affine(mu, rs, gname, bname, idx):
                    s = stp.tile([P, DC], F32, tag=f"s{idx}", name=f"s{idx}")
                    nc.vector.tensor_tensor(out=s[:], in0=rs[:],
                                            in1=bias_t[gname][:], op=OP.mult)
                    tmp = stp.tile([P, DC], F32, tag=f"tmp{idx}", name=f"tmp{idx}")
                    nc.vector.tensor_tensor(out=tmp[:], in0=mu[:], in1=s[:],
                                            op=OP.mult)
                    tb = stp.tile([P, DC], F32, tag=f"tb{idx}", name=f"tb{idx}")
                    nc.vector.tensor_tensor(out=tb[:], in0=bias_t[bname][:],
                                            in1=tmp[:], op=OP.subtract)
                    return s, tb

                mu1, rs1 = bn_stats(combraw, 0)
                s1, t1 = bn_affine(mu1, rs1, "g1", "bt1", 0)
                combT = [ap_.tile([P, ns], BF16, tag=f"combT{c}", name=f"combT{c}")
                         for c in range(DC)]
                for c in range(DC):
                    nc.scalar.activation(combT[c][:], combraw[c][:],
                                         AF.Identity, bias=t1[:, c:c + 1],
                                         scale=s1[:, c:c + 1])

                r2T = [ap_.tile([P, ns], BF16, tag=f"r2T{c}", name=f"r2T{c}")
                       for c in range(DC)]
                for nb in range(NB):
                    n_sl = slice(nb * nbf, (nb + 1) * nbf)
                    tT = [mp_.tile([P, nbf], BF16, tag=f"tT{fe}", name=f"tT{fe}")
                          for fe in range(FC)]
                    for fe in range(FC):
                        ps = bps.tile([P, nbf], F32, tag="psB", name="psB")
                        for c in range(DC):
                            nc.tensor.matmul(
                                ps[:], we1_t[c][:, fe * P:(fe + 1) * P],
                                combT[c][:, n_sl],
                                start=(c == 0), stop=(c == DC - 1))
                        nc.scalar.activation(tT[fe][:], ps[:], AF.Tanh,
                                             bias=bias_t["be1"][:, fe:fe + 1])
                    for fc in range(DC):
                        ps = bps.tile([P, nbf], F32, tag="psB", name="psB")
                        for fe in range(FC):
                            nc.tensor.matmul(
                                ps[:], we2_t[fe][:, fc * P:(fc + 1) * P],
                                tT[fe][:],
                                start=(fe == 0), stop=(fe == FC - 1))
                        # r2 = (psum + b_e2) + comb  (residual, bias fused)
                        nc.vector.scalar_tensor_tensor(
                            out=r2T[fc][:, n_sl], in0=ps[:],
                            scalar=bias_t["be2"][:, fc:fc + 1],
                            in1=combT[fc][:, n_sl], op0=OP.add, op1=OP.add)

                mu2, rs2 = bn_stats(r2T, 1)
                s2, t2 = bn_affine(mu2, rs2, "g2", "bt2", 1)
                c2T = combraw  # reuse buffers
                for c in range(DC):
                    nc.scalar.activation(c2T[c][:], r2T[c][:], AF.Identity,
                                         bias=t2[:, c:c + 1],
                                         scale=s2[:, c:c + 1])

                yT = [ap_.tile([P, ns], BF16, tag=f"yT{c}", name=f"yT{c}") for c in range(DC)]
                for nb in range(NB):
                    n_sl = slice(nb * nbf, (nb + 1) * nbf)
                    for fc in range(DC):
                        ps = bps.tile([P, nbf], F32, tag="psB", name="psB")
                        for c in range(DC):
                            nc.tensor.matmul(
                                ps[:], wd_t[c][:, fc * P:(fc + 1) * P],
                                c2T[c][:, n_sl],
                                start=(c == 0), stop=(c == DC - 1))
                        nc.scalar.activation(yT[fc][:, n_sl], ps[:],
                                             AF.Identity,
                                             bias=bias_t["bd"][:, fc:fc + 1])

                mu3, rs3 = bn_stats(yT, 2)
                s3, t3 = bn_affine(mu3, rs3, "gd", "btd", 2)

                # fused BN3+tanh, transpose back to [ns, d], store
                with (
                    tc.tile_pool(name="ops", bufs=2, space="PSUM") as opsp,
                    tc.tile_pool(name="onat", bufs=3) as onp,
                ):
                    for t in range(T):
                        otmp = onp.tile([P, d], F32, tag="otmp", name="otmp")
                        for j in range(DC):
                            nc.scalar.activation(
                                otmp[:, j * P:(j + 1) * P],
                                yT[j][:, t * P:(t + 1) * P], AF.Tanh,
                                bias=t3[:, j:j + 1], scale=s3[:, j:j + 1])
                        tps = opsp.tile([P, d], F32, name="otps")
                        for j in range(DC):
                            nc.tensor.transpose(
                                out=tps[:, j * P:(j + 1) * P],
                                in_=otmp[:, j * P:(j + 1) * P],
                                identity=ident[:])
                        onat = onp.tile([P, d], F32, tag="onat", name="onat")
                        nc.scalar.copy(onat[:], tps[:])
                        nc.sync.dma_start(out=out[t * P:(t + 1) * P, :],
                                          in_=onat[:])

    nc.finalize()
    return nc


def _split_bf16(x):
    h = x.astype(ml_dtypes.bfloat16)
    l = (x - h.astype(np.float32)).astype(ml_dtypes.bfloat16)
    return h, l


def _chunk_vec(v, cols):
    # [cols*128] feature vector -> [128, cols] feature-major chunk layout
    return np.ascontiguousarray(v.reshape(cols, P).T)


def prepare_inputs(src, anchor_2, W_dim, b_dim, W_fus, b_fus, W_e1, b_e1,
                   W_e2, b_e2, g1, bt1, g2, bt2, W_d, b_d, g_d, bt_d,
                   n_cores=N_CORES, ns=N_FULL // N_CORES):
    """Host-side prep: shard + transpose + bf16-split + layout transforms."""
    d = src.shape[1]
    f = W_e1.shape[1]
    DC, FC = d // P, f // P
    am2 = 0.5 * (anchor_2.astype(np.float64) ** 2).sum(1).astype(np.float32)
    am2b = np.broadcast_to(am2[None, :], (P, anchor_2.shape[0]))
    am2b = np.ascontiguousarray(am2b)
    ah, al = _split_bf16(anchor_2.T.copy())
    shared = dict(
        anchT_h=ah, anchT_l=al,
        anchor=np.ascontiguousarray(anchor_2),
        am2b=am2b,
        wdim=(W_dim / K).astype(ml_dtypes.bfloat16),
        wfus=W_fus.astype(ml_dtypes.bfloat16),
        we1=W_e1.astype(ml_dtypes.bfloat16),
        we2=W_e2.astype(ml_dtypes.bfloat16),
        wd=W_d.astype(ml_dtypes.bfloat16),
        bdim=_chunk_vec(b_dim, DC), bfus=_chunk_vec(b_fus, DC),
        be1=_chunk_vec(b_e1, FC), be2=_chunk_vec(b_e2, DC),
        bd=_chunk_vec(b_d, DC),
        g1=_chunk_vec(g1, DC), bt1=_chunk_vec(bt1, DC),
        g2=_chunk_vec(g2, DC), bt2=_chunk_vec(bt2, DC),
        gd=_chunk_vec(g_d, DC), btd=_chunk_vec(bt_d, DC),
    )
    in_maps = []
    for c in range(n_cores):
        shard = src[c * ns:(c + 1) * ns].T.copy()   # [d, ns]
        sh, sl = _split_bf16(shard)
        in_maps.append(dict(shared, srcT_h=sh, srcT_l=sl))
    return in_maps


_NC_CACHE = {}


def kernel(**inputs):
    key = "full"
    if key not in _NC_CACHE:
        _NC_CACHE[key] = build_kernel()
    nc = _NC_CACHE[key]
    in_maps = prepare_inputs(**{k: np.asarray(v) for k, v in inputs.items()})
    res = run_bass_kernel_spmd(nc, in_maps, core_ids=list(range(N_CORES)))
    return np.concatenate([r["out"] for r in res.results], axis=0)



# revision 2
# speedup vs baseline: 1.4280x; 1.4280x over previous
"""Trainium2 Bass kernel for nn_Encoder_61830349193463 (retrieval_knn).

v2 strategy (data-parallel over src rows, 8 NeuronCores):
  - Each core gets a 2048-row shard of src; anchors + weights replicated.
  - kNN sims via a SINGLE bf16 PE matmul pass (vs 3-pass split-bf16 in v1):
      sim = sh.ah + (256 - 0.5*||a||^2)  [bias folded in as 2 extra split-bf16
      contraction rows via a K=4 ones matmul]
    Approx sim noise ~0.07 vs typical top-5/6 gap ~4, so the approx global
    top-6 contains the true top-5 for all but ~8/16384 rows (validated
    host-side); those few rows are within the rel-err budget.
  - Per (tile, quarter): max8 + find_index8 run DIRECTLY on the [128,2048]
    f32 PSUM tile (no drain).  Quarter top-8s merged to global top-6 by
    value matching.
  - Top-6 anchors gathered fp32; ranks 4..6 re-scored EXACTLY as
    d2 = sum((s-g)^2): DVE subtract + ScalarE Square-accumulate.  Best 2
    of the 3 refined + unconditional top-3 = exact top-5; mask built via
    is_le against the 4th-largest of a padded 8-slot score row.
  - mean(top5) = sum_k w_k * G_k via 6 chained DVE stt ops (w in {0,1},
    the /K folded into W_dim host-side); transpose to feature-major
    deferred to phase B (PSUM is fully owned by the scans in phase A).
  - Dense chain (same as v1): feature-major, BN stats AllReduced (3 tiny
    collectives), all matmuls bf16.
"""

import numpy as np

import concourse.bacc as bacc
import concourse.bass as bass
import concourse.mybir as mybir
import concourse.tile as tile
from concourse.bass import IndirectOffsetOnAxis
from concourse.bass_utils import run_bass_kernel_spmd
from concourse.masks import make_identity
import ml_dtypes

F32 = mybir.dt.float32
BF16 = mybir.dt.bfloat16
U32 = mybir.dt.uint32
AF = mybir.ActivationFunctionType
OP = mybir.AluOpType
P = 128

# problem sizes (hardcoded per contract)
N_FULL, M, D, F = 16384, 8192, 512, 2048
N_CORES = 8
K = 5
EPS = 1e-5
CAND = 6            # gathered candidates; ranks 4..6 exactly re-scored


def build_kernel(ns=N_FULL // N_CORES, m=M, d=D, f=F, n_cores=N_CORES,
                 mc_free=512, q_div=4):
    """Build the SPMD Bass module. ns/m/d/f sizes are per-core."""
    DC = d // P          # contraction chunks of the d dim (4)
    FC = f // P          # chunks of the hidden dim (16)
    T = ns // P          # n-tiles per core (16)
    nbf = min(mc_free, ns)
    NB = ns // nbf       # n blocks of 512 for phase-B matmuls (4)
    MQ = m // q_div      # m-quarter size (2048)
    QC = MQ // mc_free   # 512-chunks per quarter (4)
    NQ = 8 * q_div       # merged candidate pool width (32)
    NTOT = float(ns * n_cores)

    nc = bacc.Bacc("TRN2", target_bir_lowering=False, debug=False,
                   num_devices=n_cores)

    def param(name, shape, dt=F32):
        return nc.declare_dram_parameter(name, list(shape), dt, isOutput=False)

    srcT_h = param("srcT_h", [d, ns], BF16)
    src_nat = param("src_nat", [ns, d], F32)
    anchT_h = param("anchT_h", [d, m], BF16)
    caug = param("caug", [4, m], BF16)             # rows: c_h, c_l, 0, 0
    anchor = param("anchor", [m, d], F32)          # natural, for the gather
    wdim = param("wdim", [d, d], BF16)             # pre-scaled by 1/K
    wfus = param("wfus", [2 * d, d], BF16)
    we1 = param("we1", [d, f], BF16)
    we2 = param("we2", [f, d], BF16)
    wd = param("wd", [d, d], BF16)
    bdim = param("bdim", [P, DC])
    bfus = param("bfus", [P, DC])
    be1 = param("be1", [P, FC])
    be2 = param("be2", [P, DC])
    bd = param("bd", [P, DC])
    g1 = param("g1", [P, DC]); bt1 = param("bt1", [P, DC])
    g2 = param("g2", [P, DC]); bt2 = param("bt2", [P, DC])
    gd = param("gd", [P, DC]); btd = param("btd", [P, DC])
    out = nc.declare_dram_parameter("out", [ns, d], F32, isOutput=True)

    # internal DRAM for the three BN-stat AllReduces
    cc_in = [nc.dram_tensor(f"cc{i}_in", [P, 2 * DC], F32) for i in range(3)]
    cc_space = "Shared" if n_cores > 4 else "Local"
    cc_out = [nc.dram_tensor(f"cc{i}_out", [P, 2 * DC], F32,
                             addr_space=cc_space) for i in range(3)]
    groups = [list(range(n_cores))]

    with tile.TileContext(nc) as tc:
        with (
            tc.tile_pool(name="persist", bufs=1) as pp,
            tc.tile_pool(name="wpool", bufs=1) as wp,
        ):
            ident = pp.tile([P, P], F32, name="ident")
            make_identity(nc, ident[:])
            ones4 = pp.tile([4, P], BF16, name="ones4")
            nc.vector.memset(ones4[:], 1.0)
            ones8 = pp.tile([P, 8], F32, name="ones8")
            nc.vector.memset(ones8[:], 1.0)
            zero512 = pp.tile([P, mc_free], F32, name="zero512")
            nc.vector.memset(zero512[:], 0.0)

            # resident bf16 source (hi split only) + per-quarter candidates
            sTh = []
            for c in range(DC):
                th = pp.tile([P, ns], BF16, tag=f"sTh{c}", name=f"sTh{c}")
                nc.sync.dma_start(out=th[:], in_=srcT_h[c * P:(c + 1) * P, :])
                sTh.append(th)

            vcand = [pp.tile([P, NQ], F32, tag=f"vc{t}", name=f"vc{t}")
                     for t in range(T)]
            icand = [pp.tile([P, NQ], F32, tag=f"ic{t}", name=f"ic{t}")
                     for t in range(T)]

            # neighbour-mean output, natural layout f32 (transposed in B)
            meanv_ctx = tc.tile_pool(name="meanv", bufs=1)
            mvp = meanv_ctx.__enter__()
            meanv = [mvp.tile([P, d], F32, tag=f"mv{t}", name=f"mv{t}")
                     for t in range(T)]

            # exact f32 source rows for the refine step (freed after A2)
            snat_ctx = tc.tile_pool(name="snat", bufs=1)
            snp = snat_ctx.__enter__()
            snat = [snp.tile([P, d], F32, tag=f"sn{t}", name=f"sn{t}")
                    for t in range(T)]
            for t in range(T):
                nc.sync.dma_start(out=snat[t][:],
                                  in_=src_nat[t * P:(t + 1) * P, :])

            # ============ PHASE A1: sims + per-quarter top-8 ============
            with (
                tc.tile_pool(name="aq", bufs=2) as aq_pool,
                tc.tile_pool(name="dps", bufs=2, space="PSUM") as dps,
                tc.tile_pool(name="tops", bufs=3) as tops,
            ):
                for q in range(q_div):
                    aqh = [aq_pool.tile([P, MQ], BF16, tag=f"aqh{c}",
                                        name=f"aqh{c}") for c in range(DC)]
                    for c in range(DC):
                        nc.sync.dma_start(
                            out=aqh[c][:],
                            in_=anchT_h[c * P:(c + 1) * P,
                                        q * MQ:(q + 1) * MQ])
                    caq = aq_pool.tile([4, MQ], BF16, tag="caq", name="caq")
                    nc.sync.dma_start(out=caq[:],
                                      in_=caug[:, q * MQ:(q + 1) * MQ])

                    for t in range(T):
                        ps = dps.tile([P, MQ], F32, name="dps")
                        n_sl = slice(t * P, (t + 1) * P)
                        for c in range(DC):
                            for mc in range(QC):
                                m_sl = slice(mc * mc_free, (mc + 1) * mc_free)
                                nc.tensor.matmul(ps[:, m_sl], sTh[c][:, n_sl],
                                                 aqh[c][:, m_sl],
                                                 start=(c == 0), stop=False)
                        for mc in range(QC):
                            m_sl = slice(mc * mc_free, (mc + 1) * mc_free)
                            nc.tensor.matmul(ps[:, m_sl], ones4[:],
                                             caq[:, m_sl],
                                             start=False, stop=True)
                        v8 = tops.tile([P, 8], F32, tag="v8", name="v8")
                        nc.vector.max(out=v8[:], in_=ps[:])
                        i8 = tops.tile([P, 8], U32, tag="i8", name="i8")
                        nc.vector.max_index(out=i8[:], in_max=v8[:],
                                            in_values=ps[:])
                        nc.vector.tensor_copy(vcand[t][:, q * 8:(q + 1) * 8],
                                              v8[:])
                        nc.vector.tensor_scalar(
                            out=icand[t][:, q * 8:(q + 1) * 8], in0=i8[:],
                            scalar1=float(q * MQ), scalar2=None, op0=OP.add)

            # ==== PHASE A2: merge, gather top-6, exact refine, mean ====
            with (
                tc.tile_pool(name="mrg", bufs=3) as mrg,
                tc.tile_pool(name="gat", bufs=2) as gat,
            ):
                for t in range(T):
                    g8 = mrg.tile([P, 8], F32, tag="g8", name="g8")
                    nc.vector.max(out=g8[:], in_=vcand[t][:])
                    # match the global top-6 values back to their m indices
                    eqm = mrg.tile([P, CAND * NQ], F32, tag="eqm", name="eqm")
                    nc.vector.tensor_tensor(
                        out=eqm[:].rearrange("p (a b) -> p a b", a=CAND),
                        in0=vcand[t][:].rearrange(
                            "p (a q) -> p a q", a=1).to_broadcast(
                            [P, CAND, NQ]),
                        in1=g8[:, 0:CAND].rearrange(
                            "p (a o) -> p a o", o=1).to_broadcast(
                            [P, CAND, NQ]),
                        op=OP.is_equal)
                    prod = mrg.tile([P, CAND * NQ], F32, tag="prod",
                                    name="prod")
                    nc.vector.tensor_tensor(
                        out=prod[:].rearrange("p (a b) -> p a b", a=CAND),
                        in0=eqm[:].rearrange("p (a b) -> p a b", a=CAND),
                        in1=icand[t][:].rearrange(
                            "p (a q) -> p a q", a=1).to_broadcast(
                            [P, CAND, NQ]),
                        op=OP.mult)
                    idx6f = mrg.tile([P, CAND], F32, tag="idx6f", name="idx6f")
                    nc.vector.tensor_reduce(
                        out=idx6f[:],
                        in_=prod[:].rearrange("p (a b) -> p a b", a=CAND),
                        axis=mybir.AxisListType.X, op=OP.add)
                    idx6 = mrg.tile([P, CAND], U32, tag="idx6", name="idx6")
                    nc.vector.tensor_copy(idx6[:], idx6f[:])

                    G = gat.tile([P, CAND * d], F32, tag="G", name="G")
                    for k in range(CAND):
                        nc.gpsimd.indirect_dma_start(
                            out=G[:, k * d:(k + 1) * d], out_offset=None,
                            in_=anchor[:],
                            in_offset=IndirectOffsetOnAxis(
                                ap=idx6[:, k:k + 1], axis=0))

                    # exact d2 for ranks 4..6 (slots 3..5); slots 0..2
                    # forced-selected (-1e30), 6..7 forced-out (+1e30)
                    d8 = mrg.tile([P, 8], F32, tag="d8", name="d8")
                    nc.vector.memset(d8[:], 1e30)
                    nc.vector.memset(d8[:, 0:3], -1e30)
                    sq = mrg.tile([P, d], BF16, tag="sq", name="sq")
                    for k in range(3, CAND):
                        diff = mrg.tile([P, d], F32, tag="diff", name="diff")
                        nc.vector.tensor_tensor(
                            out=diff[:], in0=snat[t][:],
                            in1=G[:, k * d:(k + 1) * d], op=OP.subtract)
                        nc.scalar.activation(sq[:], diff[:], AF.Square,
                                             accum_out=d8[:, k:k + 1])
                    m8 = mrg.tile([P, 8], F32, tag="m8", name="m8")
                    nc.vector.max(out=m8[:], in_=d8[:])
                    # 4th largest = 2nd smallest of the refined 3
                    w = mrg.tile([P, 8], F32, tag="w", name="w")
                    nc.vector.scalar_tensor_tensor(
                        out=w[:], in0=d8[:], scalar=m8[:, 3:4],
                        in1=ones8[:], op0=OP.is_le, op1=OP.mult)

                    accA = mrg.tile([P, d], F32, tag="accA", name="accA")
                    accB = mrg.tile([P, d], F32, tag="accB", name="accB")
                    accs = [zero512, accA, accB, accA, accB, accA]
                    for k in range(CAND):
                        dst = meanv[t] if k == CAND - 1 else accs[k + 1]
                        nc.vector.scalar_tensor_tensor(
                            out=dst[:], in0=G[:, k * d:(k + 1) * d],
                            scalar=w[:, k:k + 1], in1=accs[k][:],
                            op0=OP.mult, op1=OP.add)

            snat_ctx.__exit__(None, None, None)

            # ================= PHASE B: dense chain =================
            # transpose the neighbour means to feature-major
            neighT = [pp.tile([P, ns], BF16, tag=f"nT{c}", name=f"nT{c}")
                      for c in range(DC)]
            with tc.tile_pool(name="tps", bufs=2, space="PSUM") as tpsp:
                for t in range(T):
                    tps = tpsp.tile([P, d], F32, name="tps")
                    for j in range(DC):
                        nc.tensor.transpose(
                            out=tps[:, j * P:(j + 1) * P],
                            in_=meanv[t][:, j * P:(j + 1) * P],
                            identity=ident[:])
                    for j in range(DC):
                        nc.scalar.copy(neighT[j][:, t * P:(t + 1) * P],
                                       tps[:, j * P:(j + 1) * P])
            meanv_ctx.__exit__(None, None, None)

            def load_w(t_dram, rows, cols, tag):
                tiles = []
                for c in range(rows // P):
                    w_ = wp.tile([P, cols], BF16, tag=f"{tag}{c}",
                                 name=f"{tag}{c}")
                    nc.sync.dma_start(out=w_[:], in_=t_dram[c * P:(c + 1) * P, :])
                    tiles.append(w_)
                return tiles

            wdim_t = load_w(wdim, d, d, "wdim")
            wfus_t = load_w(wfus, 2 * d, d, "wfus")
            we1_t = load_w(we1, d, f, "we1")
            we2_t = load_w(we2, f, d, "we2")
            wd_t = load_w(wd, d, d, "wd")

            bias_t = {}
            for name, t_dram, cols in [
                    ("bdim", bdim, DC), ("bfus", bfus, DC), ("be1", be1, FC),
                    ("be2", be2, DC), ("bd", bd, DC), ("g1", g1, DC),
                    ("bt1", bt1, DC), ("g2", g2, DC), ("bt2", bt2, DC),
                    ("gd", gd, DC), ("btd", btd, DC)]:
                bt_ = wp.tile([P, cols], F32, tag=name, name=name)
                nc.sync.dma_start(out=bt_[:], in_=t_dram[:, :])
                bias_t[name] = bt_

            with (
                tc.tile_pool(name="act", bufs=1) as ap_,
                tc.tile_pool(name="mlp", bufs=1) as mp_,
                tc.tile_pool(name="bps", bufs=4, space="PSUM") as bps,
                tc.tile_pool(name="stat", bufs=1) as stp,
                tc.tile_pool(name="dram", bufs=1, space="DRAM") as _dp,
            ):
                amp_ctx = tc.tile_pool(name="amap", bufs=1)
                amp = amp_ctx.__enter__()
                amapT = [amp.tile([P, ns], BF16, tag=f"amap{c}", name=f"amap{c}")
                         for c in range(DC)]
                for nb in range(NB):
                    n_sl = slice(nb * nbf, (nb + 1) * nbf)
                    for fc in range(DC):
                        ps = bps.tile([P, nbf], F32, tag="psB", name="psB")
                        for c in range(DC):
                            nc.tensor.matmul(
                                ps[:], wdim_t[c][:, fc * P:(fc + 1) * P],
                                neighT[c][:, n_sl],
                                start=(c == 0), stop=(c == DC - 1))
                        nc.scalar.activation(amapT[fc][:, n_sl], ps[:],
                                             AF.Identity,
                                             bias=bias_t["bdim"][:, fc:fc + 1])

                combraw = [ap_.tile([P, ns], BF16, tag=f"craw{c}", name=f"craw{c}")
                           for c in range(DC)]
                for nb in range(NB):
                    n_sl = slice(nb * nbf, (nb + 1) * nbf)
                    for fc in range(DC):
                        ps = bps.tile([P, nbf], F32, tag="psB", name="psB")
                        for c in range(2 * DC):
                            rhs = sTh[c][:, n_sl] if c < DC else \
                                amapT[c - DC][:, n_sl]
                            nc.tensor.matmul(
                                ps[:], wfus_t[c][:, fc * P:(fc + 1) * P], rhs,
                                start=(c == 0), stop=(c == 2 * DC - 1))
                        nc.scalar.activation(combraw[fc][:, n_sl], ps[:],
                                             AF.Identity,
                                             bias=bias_t["bfus"][:, fc:fc + 1])

                amp_ctx.__exit__(None, None, None)

                def bn_stats(tiles, idx):
                    st = stp.tile([P, 2 * DC], F32, tag=f"st{idx}", name=f"st{idx}")
                    scr = stp.tile([P, ns], BF16, tag="sq_scratch",
                                   name="sq_scratch")
                    for c in range(DC):
                        nc.vector.tensor_reduce(out=st[:, c:c + 1],
                                                in_=tiles[c][:],
                                                axis=mybir.AxisListType.X,
                                                op=OP.add)
                        nc.scalar.activation(scr[:], tiles[c][:], AF.Square,
                                             accum_out=st[:, DC + c:DC + c + 1])
                    nc.sync.dma_start(out=cc_in[idx][:], in_=st[:])
                    nc.gpsimd.collective_compute(
                        "AllReduce", OP.add, replica_groups=groups,
                        ins=[cc_in[idx].ap()], outs=[cc_out[idx].ap()])
                    gst = stp.tile([P, 2 * DC], F32, tag=f"gst{idx}", name=f"gst{idx}")
                    nc.sync.dma_start(out=gst[:], in_=cc_out[idx][:])
                    # mu, var=E[x^2]-mu^2, s=g/sqrt(var+eps), t=beta-mu*s
                    mu = stp.tile([P, DC], F32, tag=f"mu{idx}", name=f"mu{idx}")
                    nc.vector.tensor_scalar(out=mu[:], in0=gst[:, :DC],
                                            scalar1=1.0 / NTOT, scalar2=None,
                                            op0=OP.mult)
                    musq = stp.tile([P, DC], F32, tag=f"musq{idx}", name=f"musq{idx}")
                    nc.vector.tensor_tensor(out=musq[:], in0=mu[:], in1=mu[:],
                                            op=OP.mult)
                    var = stp.tile([P, DC], F32, tag=f"var{idx}", name=f"var{idx}")
                    nc.vector.scalar_tensor_tensor(
                        out=var[:], in0=gst[:, DC:], scalar=1.0 / NTOT,
                        in1=musq[:], op0=OP.mult, op1=OP.subtract)
                    sd = stp.tile([P, DC], F32, tag=f"sd{idx}", name=f"sd{idx}")
                    nc.vector.tensor_scalar(out=sd[:], in0=var[:], scalar1=EPS,
                                            scalar2=None, op0=OP.add)
                    nc.scalar.sqrt(sd[:], sd[:])
                    rs = stp.tile([P, DC], F32, tag=f"rs{idx}", name=f"rs{idx}")
                    nc.vector.reciprocal(rs[:], sd[:])
                    return mu, rs

                def bn_affine(mu, rs, gname, bname, idx):
                    s = stp.tile([P, DC], F32, tag=f"s{idx}", name=f"s{idx}")
                    nc.vector.tensor_tensor(out=s[:], in0=rs[:],
                                            in1=bias_t[gname][:], op=OP.mult)
                    tmp = stp.tile([P, DC], F32, tag=f"tmp{idx}", name=f"tmp{idx}")
                    nc.vector.tensor_tensor(out=tmp[:], in0=mu[:], in1=s[:],
                                            op=OP.mult)
                    tb = stp.tile([P, DC], F32, tag=f"tb{idx}", name=f"tb{idx}")
                    nc.vector.tensor_tensor(out=tb[:], in0=bias_t[bname][:],
                                            in1=tmp[:], op=OP.subtract)
                    return s, tb

                mu1, rs1 = bn_stats(combraw, 0)
                s1, t1 = bn_affine(mu1, rs1, "g1", "bt1", 0)
                combT = [ap_.tile([P, ns], BF16, tag=f"combT{c}", name=f"combT{c}")
                         for c in range(DC)]
                for c in range(DC):
                    nc.scalar.activation(combT[c][:], combraw[c][:],
                                         AF.Identity, bias=t1[:, c:c + 1],
                                         scale=s1[:, c:c + 1])

                r2T = [ap_.tile([P, ns], BF16, tag=f"r2T{c}", name=f"r2T{c}")
                       for c in range(DC)]
                for nb in range(NB):
                    n_sl = slice(nb * nbf, (nb + 1) * nbf)
                    tT = [mp_.tile([P, nbf], BF16, tag=f"tT{fe}", name=f"tT{fe}")
                          for fe in range(FC)]
                    for fe in range(FC):
                        ps = bps.tile([P, nbf], F32, tag="psB", name="psB")
                        for c in range(DC):
                            nc.tensor.matmul(
                                ps[:], we1_t[c][:, fe * P:(fe + 1) * P],
                                combT[c][:, n_sl],
                                start=(c == 0), stop=(c == DC - 1))
                        nc.scalar.activation(tT[fe][:], ps[:], AF.Tanh,
                                             bias=bias_t["be1"][:, fe:fe + 1])
                    for fc in range(DC):
                        ps = bps.tile([P, nbf], F32, tag="psB", name="psB")
                        for fe in range(FC):
                            nc.tensor.matmul(
                                ps[:], we2_t[fe][:, fc * P:(fc + 1) * P],
                                tT[fe][:],
                                start=(fe == 0), stop=(fe == FC - 1))
                        # r2 = (psum + b_e2) + comb  (residual, bias fused)
                        nc.vector.scalar_tensor_tensor(
                            out=r2T[fc][:, n_sl], in0=ps[:],
                            scalar=bias_t["be2"][:, fc:fc + 1],
                            in1=combT[fc][:, n_sl], op0=OP.add, op1=OP.add)

                mu2, rs2 = bn_stats(r2T, 1)
                s2, t2 = bn_affine(mu2, rs2, "g2", "bt2", 1)
                c2T = combraw  # reuse buffers
                for c in range(DC):
                    nc.scalar.activation(c2T[c][:], r2T[c][:], AF.Identity,
                                         bias=t2[:, c:c + 1],
                                         scale=s2[:, c:c + 1])

                yT = [ap_.tile([P, ns], BF16, tag=f"yT{c}", name=f"yT{c}")
                      for c in range(DC)]
                for nb in range(NB):
                    n_sl = slice(nb * nbf, (nb + 1) * nbf)
                    for fc in range(DC):
                        ps = bps.tile([P, nbf], F32, tag="psB", name="psB")
                        for c in range(DC):
                            nc.tensor.matmul(
                                ps[:], wd_t[c][:, fc * P:(fc + 1) * P],
                                c2T[c][:, n_sl],
                                start=(c == 0), stop=(c == DC - 1))
                        nc.scalar.activation(yT[fc][:, n_sl], ps[:],
                                             AF.Identity,
                                             bias=bias_t["bd"][:, fc:fc + 1])

                mu3, rs3 = bn_stats(yT, 2)
                s3, t3 = bn_affine(mu3, rs3, "gd", "btd", 2)

                # fused BN3+tanh, transpose back to [ns, d], store
                with (
                    tc.tile_pool(name="ops", bufs=2, space="PSUM") as opsp,
                    tc.tile_pool(name="onat", bufs=3) as onp,
                ):
                    for t in range(T):
                        otmp = onp.tile([P, d], F32, tag="otmp", name="otmp")
                        for j in range(DC):
                            nc.scalar.activation(
                                otmp[:, j * P:(j + 1) * P],
                                yT[j][:, t * P:(t + 1) * P], AF.Tanh,
                                bias=t3[:, j:j + 1], scale=s3[:, j:j + 1])
                        tps = opsp.tile([P, d], F32, name="otps")
                        for j in range(DC):
                            nc.tensor.transpose(
                                out=tps[:, j * P:(j + 1) * P],
                                in_=otmp[:, j * P:(j + 1) * P],
                                identity=ident[:])
                        onat = onp.tile([P, d], F32, tag="onat", name="onat")
                        nc.scalar.copy(onat[:], tps[:])
                        nc.sync.dma_start(out=out[t * P:(t + 1) * P, :],
                                          in_=onat[:])

    nc.finalize()
    return nc


def _chunk_vec(v, cols):
    # [cols*128] feature vector -> [128, cols] feature-major chunk layout
    return np.ascontiguousarray(v.reshape(cols, P).T)


def prepare_inputs(src, anchor_2, W_dim, b_dim, W_fus, b_fus, W_e1, b_e1,
                   W_e2, b_e2, g1, bt1, g2, bt2, W_d, b_d, g_d, bt_d,
                   n_cores=N_CORES, ns=N_FULL // N_CORES):
    """Host-side prep: shard + transpose + bf16 casts + layout transforms."""
    d = src.shape[1]
    f = W_e1.shape[1]
    m = anchor_2.shape[0]
    DC, FC = d // P, f // P
    am2 = (anchor_2.astype(np.float64) ** 2).sum(1)
    c = 256.0 - 0.5 * am2
    ch = c.astype(np.float32).astype(ml_dtypes.bfloat16)
    cl = (c - ch.astype(np.float64)).astype(np.float32).astype(
        ml_dtypes.bfloat16)
    caug = np.zeros((4, m), dtype=ml_dtypes.bfloat16)
    caug[0] = ch
    caug[1] = cl
    shared = dict(
        anchT_h=anchor_2.T.astype(ml_dtypes.bfloat16),
        caug=caug,
        anchor=np.ascontiguousarray(anchor_2),
        wdim=(W_dim / K).astype(ml_dtypes.bfloat16),
        wfus=W_fus.astype(ml_dtypes.bfloat16),
        we1=W_e1.astype(ml_dtypes.bfloat16),
        we2=W_e2.astype(ml_dtypes.bfloat16),
        wd=W_d.astype(ml_dtypes.bfloat16),
        bdim=_chunk_vec(b_dim, DC), bfus=_chunk_vec(b_fus, DC),
        be1=_chunk_vec(b_e1, FC), be2=_chunk_vec(b_e2, DC),
        bd=_chunk_vec(b_d, DC),
        g1=_chunk_vec(g1, DC), bt1=_chunk_vec(bt1, DC),
        g2=_chunk_vec(g2, DC), bt2=_chunk_vec(bt2, DC),
        gd=_chunk_vec(g_d, DC), btd=_chunk_vec(bt_d, DC),
    )
    in_maps = []
    for cix in range(n_cores):
        shard = np.ascontiguousarray(src[cix * ns:(cix + 1) * ns])
        in_maps.append(dict(
            shared,
            srcT_h=shard.T.astype(ml_dtypes.bfloat16),
            src_nat=shard.astype(np.float32)))
    return in_maps


_NC_CACHE = {}


def kernel(**inputs):
    key = "full"
    if key not in _NC_CACHE:
        _NC_CACHE[key] = build_kernel()
    nc = _NC_CACHE[key]
    in_maps = prepare_inputs(**{k: np.asarray(v) for k, v in inputs.items()})
    res = run_bass_kernel_spmd(nc, in_maps, core_ids=list(range(N_CORES)))
    return np.concatenate([r["out"] for r in res.results], axis=0)


# revision 7
# speedup vs baseline: 1.4943x; 1.0464x over previous
"""Trainium2 Bass kernel for nn_Encoder_61830349193463 (retrieval_knn).

v3 strategy (data-parallel over src rows, 8 NeuronCores):
  - Each core gets a 2048-row shard of src; anchors + weights replicated.
  - kNN sims via a SINGLE bf16 PE matmul pass:
      sim = sh.ah + (256 - 0.5*||a||^2)  [bias folded in as split-bf16
      rows of a K=4 ones matmul]
  - All 4 anchor chunks ([128, 8192] bf16 each) stay RESIDENT in SBUF, so
    the loop runs tile-major: for each 128-row tile, 4 quarter matmuls into
    PSUM with max8 + find_index8 scans DIRECTLY on PSUM, then the
    candidate post-processing (merge -> gather -> exact refine -> top-5
    mask -> mean) runs inline and pipelines against the next tile's
    matmuls/scans (keeps the PE warm; v2 serialized this after all sims).
  - Top-6 anchors gathered fp32; ranks 4..6 re-scored EXACTLY as
    d2 = sum((s-g)^2): DVE subtract + ScalarE Square-accumulate; best 2 of
    the refined 3 + unconditional top-3 = exact top-5 (mask via is_le
    against the 4th-largest of a padded 8-slot score row).
  - mean(top5) = sum_k w_k * G_k via 6 chained DVE stt ops (w in {0,1},
    the /K folded into W_dim host-side); transpose to feature-major
    deferred to phase B (PSUM fully owned by the sims in phase A).
  - Dense chain: feature-major, BN stats AllReduced (3 tiny collectives),
    all matmuls bf16.
"""

import numpy as np

import concourse.bacc as bacc
import concourse.bass as bass
import concourse.mybir as mybir
import concourse.tile as tile
from concourse.bass import IndirectOffsetOnAxis
from concourse.bass_utils import run_bass_kernel_spmd
from concourse.masks import make_identity
import ml_dtypes

F32 = mybir.dt.float32
BF16 = mybir.dt.bfloat16
U32 = mybir.dt.uint32
AF = mybir.ActivationFunctionType
OP = mybir.AluOpType
P = 128

# problem sizes (hardcoded per contract)
N_FULL, M, D, F = 16384, 8192, 512, 2048
N_CORES = 8
K = 5
EPS = 1e-5
CAND = 6            # gathered candidates; ranks 4..6 exactly re-scored


def build_kernel(ns=N_FULL // N_CORES, m=M, d=D, f=F, n_cores=N_CORES,
                 mc_free=512, q_div=4):
    """Build the SPMD Bass module. ns/m/d/f sizes are per-core."""
    DC = d // P          # contraction chunks of the d dim (4)
    FC = f // P          # chunks of the hidden dim (16)
    T = ns // P          # n-tiles per core (16)
    nbf = min(mc_free, ns)
    NB = ns // nbf       # n blocks of 512 for phase-B matmuls (4)
    MQ = m // q_div      # m-quarter size (2048)
    QC = MQ // mc_free   # 512-chunks per quarter (4)
    NQ = 8 * q_div       # merged candidate pool width (32)
    NTOT = float(ns * n_cores)

    nc = bacc.Bacc("TRN2", target_bir_lowering=False, debug=False,
                   num_devices=n_cores)

    def param(name, shape, dt=F32):
        return nc.declare_dram_parameter(name, list(shape), dt, isOutput=False)

    srcT_h = param("srcT_h", [d, ns], BF16)
    src_nat = param("src_nat", [ns, d], F32)
    anchT_h = param("anchT_h", [d, m], BF16)
    caug = param("caug", [4, m], BF16)             # rows: c_h, c_l, 0, 0
    anchor = param("anchor", [m, d], F32)          # natural, for the gather
    wdim = param("wdim", [d, d], BF16)             # pre-scaled by 1/K
    wfus = param("wfus", [2 * d, d], BF16)
    we1 = param("we1", [d, f], BF16)
    we2 = param("we2", [f, d], BF16)
    wd = param("wd", [d, d], BF16)
    bdim = param("bdim", [P, DC])
    bfus = param("bfus", [P, DC])
    be1 = param("be1", [P, FC])
    be2 = param("be2", [P, DC])
    bd = param("bd", [P, DC])
    g1 = param("g1", [P, DC]); bt1 = param("bt1", [P, DC])
    g2 = param("g2", [P, DC]); bt2 = param("bt2", [P, DC])
    gd = param("gd", [P, DC]); btd = param("btd", [P, DC])
    out = nc.declare_dram_parameter("out", [ns, d], F32, isOutput=True)

    # internal DRAM for the three BN-stat AllReduces
    cc_in = [nc.dram_tensor(f"cc{i}_in", [P, 2 * DC], F32) for i in range(3)]
    cc_space = "Shared" if n_cores > 4 else "Local"
    cc_out = [nc.dram_tensor(f"cc{i}_out", [P, 2 * DC], F32,
                             addr_space=cc_space) for i in range(3)]
    groups = [list(range(n_cores))]

    with tile.TileContext(nc) as tc:
        with (
            tc.tile_pool(name="persist", bufs=1) as pp,
            tc.tile_pool(name="wpool", bufs=1) as wp,
        ):
            ident = pp.tile([P, P], F32, name="ident")
            make_identity(nc, ident[:])
            identb = pp.tile([P, P], BF16, name="identb")
            nc.vector.tensor_copy(identb[:], ident[:])
            ones4 = pp.tile([4, P], BF16, name="ones4")
            nc.vector.memset(ones4[:], 1.0)
            ones8 = pp.tile([P, 8], F32, name="ones8")
            nc.vector.memset(ones8[:], 1.0)

            # resident bf16 source (hi split only)
            sTh = []
            for c in range(DC):
                th = pp.tile([P, ns], BF16, tag=f"sTh{c}", name=f"sTh{c}")
                nc.sync.dma_start(out=th[:], in_=srcT_h[c * P:(c + 1) * P, :])
                sTh.append(th)

            # neighbour means go to a DRAM scratch (read back in phase B)
            meanv_dram = nc.dram_tensor("meanv_scratch", [ns, d], BF16)

            # ================= PHASE A: kNN =================
            anch_ctx = tc.tile_pool(name="anch", bufs=1)
            anp = anch_ctx.__enter__()
            aT = []
            for c in range(DC):
                at = anp.tile([P, m], BF16, tag=f"aT{c}", name=f"aT{c}")
                nc.sync.dma_start(out=at[:], in_=anchT_h[c * P:(c + 1) * P, :])
                aT.append(at)
            caugt = anp.tile([4, m], BF16, tag="caug", name="caug")
            nc.sync.dma_start(out=caugt[:], in_=caug[:, :])

            with (
                tc.tile_pool(name="dps", bufs=2, space="PSUM") as dps,
                tc.tile_pool(name="tops", bufs=3) as tops,
                tc.tile_pool(name="mrg", bufs=2) as mrg,
                tc.tile_pool(name="gat", bufs=2) as gat,
                tc.tile_pool(name="snp", bufs=2) as snp,
            ):
                for t in range(T):
                    n_sl = slice(t * P, (t + 1) * P)
                    sn = snp.tile([P, d], F32, tag="sn", name="sn")
                    nc.sync.dma_start(out=sn[:],
                                      in_=src_nat[t * P:(t + 1) * P, :])
                    vcand = mrg.tile([P, NQ], F32, tag="vcand", name="vcand")
                    icand = mrg.tile([P, NQ], F32, tag="icand", name="icand")
                    for q in range(q_div):
                        ps = dps.tile([P, MQ], F32, name="dps")
                        for c in range(DC):
                            for mc in range(QC):
                                m_sl = slice(q * MQ + mc * mc_free,
                                             q * MQ + (mc + 1) * mc_free)
                                p_sl = slice(mc * mc_free, (mc + 1) * mc_free)
                                nc.tensor.matmul(ps[:, p_sl], sTh[c][:, n_sl],
                                                 aT[c][:, m_sl],
                                                 start=(c == 0), stop=False)
                        for mc in range(QC):
                            m_sl = slice(q * MQ + mc * mc_free,
                                         q * MQ + (mc + 1) * mc_free)
                            p_sl = slice(mc * mc_free, (mc + 1) * mc_free)
                            nc.tensor.matmul(ps[:, p_sl], ones4[:],
                                             caugt[:, m_sl],
                                             start=False, stop=True)
                        v8 = tops.tile([P, 8], F32, tag="v8", name="v8")
                        nc.vector.max(out=v8[:], in_=ps[:])
                        i8 = tops.tile([P, 8], U32, tag="i8", name="i8")
                        nc.vector.max_index(out=i8[:], in_max=v8[:],
                                            in_values=ps[:])
                        nc.vector.tensor_copy(vcand[:, q * 8:(q + 1) * 8],
                                              v8[:])
                        nc.vector.tensor_scalar(
                            out=icand[:, q * 8:(q + 1) * 8], in0=i8[:],
                            scalar1=float(q * MQ), scalar2=None, op0=OP.add)

                    # ---- merge quarters to global top-6 ----
                    g8 = mrg.tile([P, 8], F32, tag="g8", name="g8")
                    nc.vector.max(out=g8[:], in_=vcand[:])
                    eqm = mrg.tile([P, CAND * NQ], F32, tag="eqm", name="eqm")
                    nc.vector.tensor_tensor(
                        out=eqm[:].rearrange("p (a b) -> p a b", a=CAND),
                        in0=vcand[:].rearrange(
                            "p (a q) -> p a q", a=1).to_broadcast(
                            [P, CAND, NQ]),
                        in1=g8[:, 0:CAND].rearrange(
                            "p (a o) -> p a o", o=1).to_broadcast(
                            [P, CAND, NQ]),
                        op=OP.is_equal)
                    prod = mrg.tile([P, CAND * NQ], F32, tag="prod",
                                    name="prod")
                    nc.vector.tensor_tensor(
                        out=prod[:].rearrange("p (a b) -> p a b", a=CAND),
                        in0=eqm[:].rearrange("p (a b) -> p a b", a=CAND),
                        in1=icand[:].rearrange(
                            "p (a q) -> p a q", a=1).to_broadcast(
                            [P, CAND, NQ]),
                        op=OP.mult)
                    idx6f = mrg.tile([P, CAND], F32, tag="idx6f", name="idx6f")
                    nc.vector.tensor_reduce(
                        out=idx6f[:],
                        in_=prod[:].rearrange("p (a b) -> p a b", a=CAND),
                        axis=mybir.AxisListType.X, op=OP.add)
                    idx6 = mrg.tile([P, CAND], U32, tag="idx6", name="idx6")
                    nc.vector.tensor_copy(idx6[:], idx6f[:])

                    G = gat.tile([P, CAND * d], F32, tag="G", name="G")
                    for k in range(CAND):
                        nc.gpsimd.indirect_dma_start(
                            out=G[:, k * d:(k + 1) * d], out_offset=None,
                            in_=anchor[:],
                            in_offset=IndirectOffsetOnAxis(
                                ap=idx6[:, k:k + 1], axis=0))

                    # exact d2 for ranks 4..6 (slots 3..5); slots 0..2
                    # forced-selected (-1e30), 6..7 forced-out (+1e30)
                    d8 = mrg.tile([P, 8], F32, tag="d8", name="d8")
                    nc.vector.memset(d8[:], 1e30)
                    nc.vector.memset(d8[:, 0:3], -1e30)
                    accA = mrg.tile([P, d], F32, tag="accA", name="accA")
                    accB = mrg.tile([P, d], F32, tag="accB", name="accB")
                    sqdump = accB[:].bitcast(BF16)[:, 0:d]
                    for k in range(3, CAND):
                        diff = mrg.tile([P, d], F32, tag="diff", name="diff")
                        nc.vector.tensor_tensor(
                            out=diff[:], in0=sn[:],
                            in1=G[:, k * d:(k + 1) * d], op=OP.subtract)
                        nc.scalar.activation(sqdump, diff[:], AF.Square,
                                             accum_out=d8[:, k:k + 1])
                    m8 = mrg.tile([P, 8], F32, tag="m8", name="m8")
                    nc.vector.max(out=m8[:], in_=d8[:])
                    # 4th largest = 2nd smallest of the refined 3
                    w = mrg.tile([P, 8], F32, tag="w", name="w")
                    nc.vector.scalar_tensor_tensor(
                        out=w[:], in0=d8[:], scalar=m8[:, 3:4],
                        in1=ones8[:], op0=OP.is_le, op1=OP.mult)

                    mvt = mrg.tile([P, d], BF16, tag="mvt", name="mvt")
                    nc.vector.tensor_tensor(
                        out=accA[:].rearrange("p (a b) -> p a b", a=1),
                        in0=G[:, 0:d].rearrange("p (a b) -> p a b", a=1),
                        in1=w[:, 0:1].rearrange(
                            "p (a o) -> p a o", o=1).to_broadcast([P, 1, d]),
                        op=OP.mult)
                    accs = [None, accA, accB, accA, accB, accA]
                    for k in range(1, CAND):
                        dst = mvt if k == CAND - 1 else accs[k + 1]
                        nc.vector.scalar_tensor_tensor(
                            out=dst[:], in0=G[:, k * d:(k + 1) * d],
                            scalar=w[:, k:k + 1], in1=accs[k][:],
                            op0=OP.mult, op1=OP.add)
                    nc.sync.dma_start(out=meanv_dram[t * P:(t + 1) * P, :],
                                      in_=mvt[:])

            anch_ctx.__exit__(None, None, None)

            # ================= PHASE B: dense chain =================
            # transpose the neighbour means to feature-major
            neighT = [pp.tile([P, ns], BF16, tag=f"nT{c}", name=f"nT{c}")
                      for c in range(DC)]
            with (
                tc.tile_pool(name="tps", bufs=2, space="PSUM") as tpsp,
                tc.tile_pool(name="mvload", bufs=2) as mvl,
            ):
                for t in range(T):
                    mvt = mvl.tile([P, d], BF16, tag="mv", name="mv")
                    nc.sync.dma_start(out=mvt[:],
                                      in_=meanv_dram[t * P:(t + 1) * P, :])
                    tps = tpsp.tile([P, d], BF16, name="tps")
                    for j in range(DC):
                        nc.tensor.transpose(
                            out=tps[:, j * P:(j + 1) * P],
                            in_=mvt[:, j * P:(j + 1) * P],
                            identity=identb[:])
                    for j in range(DC):
                        nc.scalar.copy(neighT[j][:, t * P:(t + 1) * P],
                                       tps[:, j * P:(j + 1) * P])

            def load_w(t_dram, rows, cols, tag):
                tiles = []
                for c in range(rows // P):
                    w_ = wp.tile([P, cols], BF16, tag=f"{tag}{c}",
                                 name=f"{tag}{c}")
                    nc.sync.dma_start(out=w_[:], in_=t_dram[c * P:(c + 1) * P, :])
                    tiles.append(w_)
                return tiles

            wdim_t = load_w(wdim, d, d, "wdim")
            wfus_t = load_w(wfus, 2 * d, d, "wfus")
            we1_t = load_w(we1, d, f, "we1")
            we2_t = load_w(we2, f, d, "we2")
            wd_t = load_w(wd, d, d, "wd")

            bias_t = {}
            for name, t_dram, cols in [
                    ("bdim", bdim, DC), ("bfus", bfus, DC), ("be1", be1, FC),
                    ("be2", be2, DC), ("bd", bd, DC), ("g1", g1, DC),
                    ("bt1", bt1, DC), ("g2", g2, DC), ("bt2", bt2, DC),
                    ("gd", gd, DC), ("btd", btd, DC)]:
                bt_ = wp.tile([P, cols], F32, tag=name, name=name)
                nc.sync.dma_start(out=bt_[:], in_=t_dram[:, :])
                bias_t[name] = bt_

            with (
                tc.tile_pool(name="act", bufs=1) as ap_,
                tc.tile_pool(name="mlp", bufs=1) as mp_,
                tc.tile_pool(name="bps", bufs=4, space="PSUM") as bps,
                tc.tile_pool(name="stat", bufs=1) as stp,
                tc.tile_pool(name="dram", bufs=1, space="DRAM") as _dp,
            ):
                amp_ctx = tc.tile_pool(name="amap", bufs=1)
                amp = amp_ctx.__enter__()
                amapT = [amp.tile([P, ns], BF16, tag=f"amap{c}", name=f"amap{c}")
                         for c in range(DC)]
                for nb in range(NB):
                    n_sl = slice(nb * nbf, (nb + 1) * nbf)
                    for fc in range(DC):
                        ps = bps.tile([P, nbf], F32, tag="psB", name="psB")
                        for c in range(DC):
                            nc.tensor.matmul(
                                ps[:], wdim_t[c][:, fc * P:(fc + 1) * P],
                                neighT[c][:, n_sl],
                                start=(c == 0), stop=(c == DC - 1))
                        nc.scalar.activation(amapT[fc][:, n_sl], ps[:],
                                             AF.Identity,
                                             bias=bias_t["bdim"][:, fc:fc + 1])

                combraw = [ap_.tile([P, ns], BF16, tag=f"craw{c}", name=f"craw{c}")
                           for c in range(DC)]
                for nb in range(NB):
                    n_sl = slice(nb * nbf, (nb + 1) * nbf)
                    for fc in range(DC):
                        ps = bps.tile([P, nbf], F32, tag="psB", name="psB")
                        for c in range(2 * DC):
                            rhs = sTh[c][:, n_sl] if c < DC else \
                                amapT[c - DC][:, n_sl]
                            nc.tensor.matmul(
                                ps[:], wfus_t[c][:, fc * P:(fc + 1) * P], rhs,
                                start=(c == 0), stop=(c == 2 * DC - 1))
                        nc.scalar.activation(combraw[fc][:, n_sl], ps[:],
                                             AF.Identity,
                                             bias=bias_t["bfus"][:, fc:fc + 1])

                amp_ctx.__exit__(None, None, None)

                def bn_stats(tiles, idx):
                    st = stp.tile([P, 2 * DC], F32, tag=f"st{idx}", name=f"st{idx}")
                    scr = stp.tile([P, ns], BF16, tag="sq_scratch",
                                   name="sq_scratch")
                    for c in range(DC):
                        nc.vector.tensor_reduce(out=st[:, c:c + 1],
                                                in_=tiles[c][:],
                                                axis=mybir.AxisListType.X,
                                                op=OP.add)
                        nc.scalar.activation(scr[:], tiles[c][:], AF.Square,
                                             accum_out=st[:, DC + c:DC + c + 1])
                    nc.sync.dma_start(out=cc_in[idx][:], in_=st[:])
                    nc.gpsimd.collective_compute(
                        "AllReduce", OP.add, replica_groups=groups,
                        ins=[cc_in[idx].ap()], outs=[cc_out[idx].ap()])
                    gst = stp.tile([P, 2 * DC], F32, tag=f"gst{idx}", name=f"gst{idx}")
                    nc.sync.dma_start(out=gst[:], in_=cc_out[idx][:])
                    # mu, var=E[x^2]-mu^2, s=g/sqrt(var+eps), t=beta-mu*s
                    mu = stp.tile([P, DC], F32, tag=f"mu{idx}", name=f"mu{idx}")
                    nc.vector.tensor_scalar(out=mu[:], in0=gst[:, :DC],
                                            scalar1=1.0 / NTOT, scalar2=None,
                                            op0=OP.mult)
                    musq = stp.tile([P, DC], F32, tag=f"musq{idx}", name=f"musq{idx}")
                    nc.vector.tensor_tensor(out=musq[:], in0=mu[:], in1=mu[:],
                                            op=OP.mult)
                    var = stp.tile([P, DC], F32, tag=f"var{idx}", name=f"var{idx}")
                    nc.vector.scalar_tensor_tensor(
                        out=var[:], in0=gst[:, DC:], scalar=1.0 / NTOT,
                        in1=musq[:], op0=OP.mult, op1=OP.subtract)
                    sd = stp.tile([P, DC], F32, tag=f"sd{idx}", name=f"sd{idx}")
                    nc.vector.tensor_scalar(out=sd[:], in0=var[:], scalar1=EPS,
                                            scalar2=None, op0=OP.add)
                    nc.scalar.sqrt(sd[:], sd[:])
                    rs = stp.tile([P, DC], F32, tag=f"rs{idx}", name=f"rs{idx}")
                    nc.vector.reciprocal(rs[:], sd[:])
                    return mu, rs

                def bn_affine(mu, rs, gname, bname, idx):
                    s = stp.tile([P, DC], F32, tag=f"s{idx}", name=f"s{idx}")
                    nc.vector.tensor_tensor(out=s[:], in0=rs[:],
                                            in1=bias_t[gname][:], op=OP.mult)
                    tmp = stp.tile([P, DC], F32, tag=f"tmp{idx}", name=f"tmp{idx}")
                    nc.vector.tensor_tensor(out=tmp[:], in0=mu[:], in1=s[:],
                                            op=OP.mult)
                    tb = stp.tile([P, DC], F32, tag=f"tb{idx}", name=f"tb{idx}")
                    nc.vector.tensor_tensor(out=tb[:], in0=bias_t[bname][:],
                                            in1=tmp[:], op=OP.subtract)
                    return s, tb

                mu1, rs1 = bn_stats(combraw, 0)
                s1, t1 = bn_affine(mu1, rs1, "g1", "bt1", 0)
                combT = [ap_.tile([P, ns], BF16, tag=f"combT{c}", name=f"combT{c}")
                         for c in range(DC)]
                for c in range(DC):
                    nc.scalar.activation(combT[c][:], combraw[c][:],
                                         AF.Identity, bias=t1[:, c:c + 1],
                                         scale=s1[:, c:c + 1])

                r2T = [ap_.tile([P, ns], BF16, tag=f"r2T{c}", name=f"r2T{c}")
                       for c in range(DC)]
                for nb in range(NB):
                    n_sl = slice(nb * nbf, (nb + 1) * nbf)
                    tT = [mp_.tile([P, nbf], BF16, tag=f"tT{fe}", name=f"tT{fe}")
                          for fe in range(FC)]
                    for fe in range(FC):
                        ps = bps.tile([P, nbf], F32, tag="psB", name="psB")
                        for c in range(DC):
                            nc.tensor.matmul(
                                ps[:], we1_t[c][:, fe * P:(fe + 1) * P],
                                combT[c][:, n_sl],
                                start=(c == 0), stop=(c == DC - 1))
                        nc.scalar.activation(tT[fe][:], ps[:], AF.Tanh,
                                             bias=bias_t["be1"][:, fe:fe + 1])
                    for fc in range(DC):
                        ps = bps.tile([P, nbf], F32, tag="psB", name="psB")
                        for fe in range(FC):
                            nc.tensor.matmul(
                                ps[:], we2_t[fe][:, fc * P:(fc + 1) * P],
                                tT[fe][:],
                                start=(fe == 0), stop=(fe == FC - 1))
                        # r2 = (psum + b_e2) + comb  (residual, bias fused)
                        nc.vector.scalar_tensor_tensor(
                            out=r2T[fc][:, n_sl], in0=ps[:],
                            scalar=bias_t["be2"][:, fc:fc + 1],
                            in1=combT[fc][:, n_sl], op0=OP.add, op1=OP.add)

                mu2, rs2 = bn_stats(r2T, 1)
                s2, t2 = bn_affine(mu2, rs2, "g2", "bt2", 1)
                c2T = combraw  # reuse buffers
                for c in range(DC):
                    nc.scalar.activation(c2T[c][:], r2T[c][:], AF.Identity,
                                         bias=t2[:, c:c + 1],
                                         scale=s2[:, c:c + 1])

                yT = [ap_.tile([P, ns], BF16, tag=f"yT{c}", name=f"yT{c}")
                      for c in range(DC)]
                for nb in range(NB):
                    n_sl = slice(nb * nbf, (nb + 1) * nbf)
                    for fc in range(DC):
                        ps = bps.tile([P, nbf], F32, tag="psB", name="psB")
                        for c in range(DC):
                            nc.tensor.matmul(
                                ps[:], wd_t[c][:, fc * P:(fc + 1) * P],
                                c2T[c][:, n_sl],
                                start=(c == 0), stop=(c == DC - 1))
                        nc.scalar.activation(yT[fc][:, n_sl], ps[:],
                                             AF.Identity,
                                             bias=bias_t["bd"][:, fc:fc + 1])

                mu3, rs3 = bn_stats(yT, 2)
                s3, t3 = bn_affine(mu3, rs3, "gd", "btd", 2)

                # fused BN3+tanh, transpose back to [ns, d], store
                with (
                    tc.tile_pool(name="ops", bufs=2, space="PSUM") as opsp,
                    tc.tile_pool(name="onat", bufs=3) as onp,
                ):
                    for t in range(T):
                        otmp = onp.tile([P, d], F32, tag="otmp", name="otmp")
                        for j in range(DC):
                            nc.scalar.activation(
                                otmp[:, j * P:(j + 1) * P],
                                yT[j][:, t * P:(t + 1) * P], AF.Tanh,
                                bias=t3[:, j:j + 1], scale=s3[:, j:j + 1])
                        tps = opsp.tile([P, d], F32, name="otps")
                        for j in range(DC):
                            nc.tensor.transpose(
                                out=tps[:, j * P:(j + 1) * P],
                                in_=otmp[:, j * P:(j + 1) * P],
                                identity=ident[:])
                        onat = onp.tile([P, d], F32, tag="onat", name="onat")
                        nc.scalar.copy(onat[:], tps[:])
                        nc.sync.dma_start(out=out[t * P:(t + 1) * P, :],
                                          in_=onat[:])

    nc.finalize()
    return nc


def _chunk_vec(v, cols):
    # [cols*128] feature vector -> [128, cols] feature-major chunk layout
    return np.ascontiguousarray(v.reshape(cols, P).T)


def prepare_inputs(src, anchor_2, W_dim, b_dim, W_fus, b_fus, W_e1, b_e1,
                   W_e2, b_e2, g1, bt1, g2, bt2, W_d, b_d, g_d, bt_d,
                   n_cores=N_CORES, ns=N_FULL // N_CORES):
    """Host-side prep: shard + transpose + bf16 casts + layout transforms."""
    d = src.shape[1]
    f = W_e1.shape[1]
    m = anchor_2.shape[0]
    DC, FC = d // P, f // P
    am2 = (anchor_2.astype(np.float64) ** 2).sum(1)
    c = 256.0 - 0.5 * am2
    ch = c.astype(np.float32).astype(ml_dtypes.bfloat16)
    cl = (c - ch.astype(np.float64)).astype(np.float32).astype(
        ml_dtypes.bfloat16)
    caug = np.zeros((4, m), dtype=ml_dtypes.bfloat16)
    caug[0] = ch
    caug[1] = cl
    shared = dict(
        anchT_h=anchor_2.T.astype(ml_dtypes.bfloat16),
        caug=caug,
        anchor=np.ascontiguousarray(anchor_2),
        wdim=(W_dim / K).astype(ml_dtypes.bfloat16),
        wfus=W_fus.astype(ml_dtypes.bfloat16),
        we1=W_e1.astype(ml_dtypes.bfloat16),
        we2=W_e2.astype(ml_dtypes.bfloat16),
        wd=W_d.astype(ml_dtypes.bfloat16),
        bdim=_chunk_vec(b_dim, DC), bfus=_chunk_vec(b_fus, DC),
        be1=_chunk_vec(b_e1, FC), be2=_chunk_vec(b_e2, DC),
        bd=_chunk_vec(b_d, DC),
        g1=_chunk_vec(g1, DC), bt1=_chunk_vec(bt1, DC),
        g2=_chunk_vec(g2, DC), bt2=_chunk_vec(bt2, DC),
        gd=_chunk_vec(g_d, DC), btd=_chunk_vec(bt_d, DC),
    )
    in_maps = []
    for cix in range(n_cores):
        shard = np.ascontiguousarray(src[cix * ns:(cix + 1) * ns])
        in_maps.append(dict(
            shared,
            srcT_h=shard.T.astype(ml_dtypes.bfloat16),
            src_nat=shard.astype(np.float32)))
    return in_maps


_NC_CACHE = {}


def kernel(**inputs):
    key = "full"
    if key not in _NC_CACHE:
        _NC_CACHE[key] = build_kernel()
    nc = _NC_CACHE[key]
    in_maps = prepare_inputs(**{k: np.asarray(v) for k, v in inputs.items()})
    res = run_bass_kernel_spmd(nc, in_maps, core_ids=list(range(N_CORES)))
    return np.concatenate([r["out"] for r in res.results], axis=0)
